# revision 18
# baseline (speedup 1.0000x reference)
"""MoE (top-1 gating, DeepSpeed-style capacity) Trainium2 kernel.

Expert-parallel across 8 NeuronCores: core e owns expert e's FFN weights.
hidden_states (and its transpose) + gate weight are replicated; every core
redundantly computes the (cheap) global routing, then gathers its own
expert's tokens via indirect DMA, runs the two big GEMMs in fp32r, scales
by the gate value, and outputs its expert slab [C, H] plus the slot->token
table.  The host unshards with a permutation gather.

Self-contained: hardcodes all shapes from the problem spec.
"""

import sys

sys.path.insert(0, "/opt/trn_rl_repo")

import numpy as np

import concourse.bacc as bacc
import concourse.bass as bass
import concourse.mybir as mybir
import concourse.tile as tile
from concourse.masks import make_identity, make_upper_triangular

S, H, E, DFF = 8192, 1024, 8, 4096
CAP = (S * 1) // E  # 1024
P = 128
T = S // P          # 64 token tiles
KH = H // P         # 8 k-tiles over H
KF = DFF // P       # 32 f-tiles over DFF
CT = CAP // P       # 8 slot tiles
NC = 8              # cores
SENT = 60000        # sentinel token id for empty slots (> S)
DROP = 20000        # scatter offset for "not my token" (> CAP-1 -> bounds-dropped)

f32 = mybir.dt.float32
f32r = mybir.dt.float32r
i32 = mybir.dt.int32

_CACHE = {}
GELU_MODE = "hw"   # "hw": fused Gelu_apprx_tanh ACT op; "sim": explicit ops


def _r(ap):
    return ap.bitcast(f32r)


def build_nc(gelu_mode=None):
    """Build the per-core Bass program (identical on all cores; per-core
    behavior comes from the per-core inputs)."""
    global GELU_MODE
    if gelu_mode is not None:
        GELU_MODE = gelu_mode
    nc = bacc.Bacc(num_swdge_queues=4)

    x = nc.dram_tensor("x", [S, H], f32, kind="ExternalInput")
    xT = nc.dram_tensor("xT", [H, S], f32, kind="ExternalInput")
    wgt = nc.dram_tensor("wgt", [P, KH * E], f32, kind="ExternalInput")
    w1t = nc.dram_tensor("w1t", [KF, KH, P, P], f32r, kind="ExternalInput")
    b1t = nc.dram_tensor("b1t", [KF, P], f32, kind="ExternalInput")
    w2e = nc.dram_tensor("w2e", [DFF, H], f32r, kind="ExternalInput")
    b2e = nc.dram_tensor("b2e", [1, H], f32r, kind="ExternalInput")
    myexp = nc.dram_tensor("myexp", [P, 1], f32, kind="ExternalInput")

    eo = nc.dram_tensor("eo", [CAP, H], f32, kind="ExternalOutput")
    tok = nc.dram_tensor("tok", [CAP, 1], i32, kind="ExternalOutput")
    laux = nc.dram_tensor("laux", [1, 1], f32, kind="ExternalOutput")
    counts = nc.dram_tensor("counts", [1, E], i32, kind="ExternalOutput")

    pr_tbl = nc.dram_tensor("pr_tbl", [CAP, 2], i32)

    with tile.TileContext(nc) as tc:
        _emit(nc, tc, x, xT, wgt, w1t, b1t, w2e, b2e, myexp,
              eo, tok, laux, counts, pr_tbl)
    if not nc.is_finalized():
        nc.finalize()
    return nc


def _emit(nc, tc, x, xT, wgt, w1t, b1t, w2e, b2e, myexp,
          eo, tok, laux, counts, pr_tbl):
    from contextlib import ExitStack

    ctx = ExitStack()
    with ctx:
        const = ctx.enter_context(tc.tile_pool(name="const", bufs=1))
        # ---- constants
        ident = const.tile([P, P], f32, tag="ident")
        make_identity(nc, ident[:])
        u128 = const.tile([P, P], f32, tag="u128")
        make_upper_triangular(nc, u128[:], val=1.0, diag=True)
        ones_col = const.tile([P, 1], f32, tag="ones_col")
        nc.vector.memset(ones_col[:], 1.0)
        ones_row = const.tile([1, P], f32, tag="ones_row")
        nc.vector.memset(ones_row[:], 1.0)
        ones_row_r = const.tile([1, P], f32r, tag="ones_row_r")
        nc.vector.tensor_copy(ones_row_r[:], ones_row[:])
        zeros64 = const.tile([1, T], f32, tag="zeros64")
        nc.vector.memset(zeros64[:], 0.0)
        wg_sb = const.tile([P, KH * E], f32, tag="wg_sb")
        nc.sync.dma_start(out=wg_sb[:], in_=wgt[:])
        myexp_sb = const.tile([P, 1], f32, tag="myexp_sb")
        nc.sync.dma_start(out=myexp_sb[:], in_=myexp[:])
        b1_sb = const.tile([P, KF], f32, tag="b1_sb")
        nc.sync.dma_start(out=b1_sb[:], in_=b1t.rearrange("f p -> p f"))
        b2_sb = const.tile([1, H], f32r, tag="b2_sb")
        nc.sync.dma_start(out=b2_sb[:], in_=b2e[:])
        # iota over e within (t, e) free layout -> int then float
        iota_e_i = const.tile([P, T * E], i32, tag="iota_e_i")
        nc.gpsimd.iota(iota_e_i[:].rearrange("p (t e) -> p t e", e=E),
                       pattern=[[0, T], [1, E]], base=0, channel_multiplier=0)
        iota_e = const.tile([P, T * E], f32, tag="iota_e")
        nc.vector.tensor_copy(iota_e[:], iota_e_i[:])
        # token ids: tokid[p, t] = t*128 + p
        tokid_i = const.tile([P, T], i32, tag="tokid_i")
        nc.gpsimd.iota(tokid_i[:], pattern=[[P, T]], base=0, channel_multiplier=1)

        # persistent-ish routing results used by the GEMM phase
        tok_sb = const.tile([P, CT], i32, tag="tok_sb")
        gate_sb = const.tile([P, CT], f32, tag="gate_sb")

        # =====================  logits: logitsT[e, s] = (wg.T @ x.T)  =====
        with tc.tile_pool(name="route_sb", bufs=1) as rsb, \
             tc.tile_pool(name="route_ps", bufs=2, space="PSUM") as rps, \
             tc.tile_pool(name="xk_pool", bufs=2) as xkp:
            logitsT = rsb.tile([E, S], f32, tag="logitsT")
            SC = 512
            for sc in range(S // SC):
                xk = xkp.tile([P, KH * SC], f32, tag="xk")
                nc.sync.dma_start(
                    out=xk[:].rearrange("p (k s) -> p k s", k=KH),
                    in_=xT[:, sc * SC:(sc + 1) * SC].rearrange(
                        "(k p) s -> p k s", p=P),
                )
                psl = rps.tile([E, SC], f32, tag="psl")
                for k in range(KH):
                    nc.tensor.matmul(
                        out=psl[:],
                        lhsT=wg_sb[:, k * E:(k + 1) * E],
                        rhs=xk[:, k * SC:(k + 1) * SC],
                        start=(k == 0), stop=(k == KH - 1),
                    )
                nc.vector.tensor_copy(logitsT[:, sc * SC:(sc + 1) * SC], psl[:])

            # ==================  transpose to token-major [p, (t e)]  =====
            ltm = rsb.tile([P, T * E], f32, tag="ltm")
            for t in range(T):
                pst = rps.tile([P, E], f32, tag="pst")
                nc.tensor.transpose(
                    out=pst[:], in_=logitsT[:, t * P:(t + 1) * P],
                    identity=ident[0:E, 0:E],
                )
                nc.vector.tensor_copy(ltm[:, t * E:(t + 1) * E], pst[:])

            # ==================  softmax / argmax / aux-loss  =============
            ltm3 = ltm[:].rearrange("p (t e) -> p t e", e=E)
            rowmax = rsb.tile([P, T], f32, tag="rowmax")
            nc.vector.tensor_reduce(rowmax[:], ltm3, axis=mybir.AxisListType.X,
                                    op=mybir.AluOpType.max)
            mask1 = rsb.tile([P, T * E], f32, tag="mask1")
            rm_b = rowmax[:].rearrange("p (t o) -> p t o", o=1).broadcast_to([P, T, E])
            nc.vector.tensor_tensor(mask1[:].rearrange("p (t e) -> p t e", e=E),
                                    ltm3, rm_b, op=mybir.AluOpType.is_equal)
            expt = rsb.tile([P, T * E], f32, tag="expt")
            nc.scalar.activation(expt[:], ltm[:], mybir.ActivationFunctionType.Exp)
            expt3 = expt[:].rearrange("p (t e) -> p t e", e=E)
            denom = rsb.tile([P, T], f32, tag="denom")
            nc.vector.tensor_reduce(denom[:], expt3, axis=mybir.AxisListType.X,
                                    op=mybir.AluOpType.add)
            grecip = rsb.tile([P, T], f32, tag="grecip")
            nc.vector.reciprocal(grecip[:], denom[:])
            gmax = rsb.tile([P, T], f32, tag="gmax")
            nc.vector.tensor_reduce(gmax[:], expt3, axis=mybir.AxisListType.X,
                                    op=mybir.AluOpType.max)
            gval = rsb.tile([P, T], f32, tag="gval")
            nc.vector.tensor_mul(gval[:], gmax[:], grecip[:])
            gates = rsb.tile([P, T * E], f32, tag="gates")
            gr_b = grecip[:].rearrange("p (t o) -> p t o", o=1).broadcast_to([P, T, E])
            nc.vector.tensor_tensor(gates[:].rearrange("p (t e) -> p t e", e=E),
                                    expt3, gr_b, op=mybir.AluOpType.mult)

            # column sums over all tokens (ones @ .) for me / ce
            ps_me = rps.tile([1, T * E], f32, tag="ps_me", bufs=1)
            nc.tensor.matmul(out=ps_me[:], lhsT=ones_col[:], rhs=gates[:],
                             start=True, stop=True)
            ps_ce = rps.tile([1, T * E], f32, tag="ps_ce", bufs=1)
            nc.tensor.matmul(out=ps_ce[:], lhsT=ones_col[:], rhs=mask1[:],
                             start=True, stop=True)
            me8 = rsb.tile([1, E], f32, tag="me8")
            nc.vector.tensor_reduce(
                me8[:], ps_me[:].rearrange("p (t e) -> p e t", e=E),
                axis=mybir.AxisListType.X, op=mybir.AluOpType.add)
            ce8 = rsb.tile([1, E], f32, tag="ce8")
            nc.vector.tensor_reduce(
                ce8[:], ps_ce[:].rearrange("p (t e) -> p e t", e=E),
                axis=mybir.AxisListType.X, op=mybir.AluOpType.add)
            prod8 = rsb.tile([1, E], f32, tag="prod8")
            nc.vector.tensor_mul(prod8[:], me8[:], ce8[:])
            laux_sb = rsb.tile([1, 1], f32, tag="laux_sb")
            nc.vector.tensor_reduce(laux_sb[:], prod8[:],
                                    axis=mybir.AxisListType.X,
                                    op=mybir.AluOpType.add)
            nc.vector.tensor_scalar_mul(laux_sb[:], laux_sb[:],
                                        float(E) / float(S) / float(S))
            nc.sync.dma_start(out=laux[:], in_=laux_sb[:])

            # ==================  global cumsum -> positions  ==============
            ps_cs = rps.tile([P, T * E], f32, tag="ps_cs", bufs=1)
            nc.tensor.matmul(out=ps_cs[:], lhsT=u128[:], rhs=mask1[:],
                             start=True, stop=True)
            # per-(t,e) totals = column sums of mask1 = ps_ce (partition 0)
            trow = rsb.tile([1, T * E], f32, tag="trow")
            nc.vector.tensor_copy(trow[:], ps_ce[0:1, :])
            offs = rsb.tile([1, T * E], f32, tag="offs")
            for e in range(E):
                nc.vector.tensor_tensor_scan(
                    out=offs[:].rearrange("p (t e) -> p e t", e=E)[:, e, :],
                    data0=trow[:].rearrange("p (t e) -> p e t", e=E)[:, e, :],
                    data1=zeros64[:],
                    initial=0.0,
                    op0=mybir.AluOpType.add, op1=mybir.AluOpType.add)
            nc.vector.tensor_sub(offs[:], offs[:], trow[:])
            nc.tensor.matmul(out=ps_cs[:], lhsT=ones_row[:], rhs=offs[:],
                             start=False, stop=True, skip_group_check=True)
            # ps_cs now = 1-based global position of each (token, expert) hit

            ltcap = rsb.tile([P, T * E], f32, tag="ltcap")
            nc.vector.tensor_scalar(ltcap[:], ps_cs[:], float(CAP), None,
                                    op0=mybir.AluOpType.is_le)
            maskF = rsb.tile([P, T * E], f32, tag="maskF")
            nc.vector.tensor_mul(maskF[:], mask1[:], ltcap[:])
            maskF3 = maskF[:].rearrange("p (t e) -> p t e", e=E)

            # exp_counts
            ps_cnt = rps.tile([1, T * E], f32, tag="ps_cnt", bufs=1)
            nc.tensor.matmul(out=ps_cnt[:], lhsT=ones_col[:], rhs=maskF[:],
                             start=True, stop=True)
            cnt8 = rsb.tile([1, E], f32, tag="cnt8")
            nc.vector.tensor_reduce(
                cnt8[:], ps_cnt[:].rearrange("p (t e) -> p e t", e=E),
                axis=mybir.AxisListType.X, op=mybir.AluOpType.add)
            cnt8i = rsb.tile([1, E], i32, tag="cnt8i")
            nc.vector.tensor_copy(cnt8i[:], cnt8[:])
            nc.sync.dma_start(out=counts[:], in_=cnt8i[:])

            # slot index / expert id / kept per token
            loc0 = rsb.tile([P, T * E], f32, tag="loc0")
            nc.vector.tensor_scalar_add(loc0[:], ps_cs[:], -1.0)
            locm = rsb.tile([P, T * E], f32, tag="locm")
            nc.vector.tensor_mul(locm[:], loc0[:], maskF[:])
            slot = rsb.tile([P, T], f32, tag="slot")
            nc.vector.tensor_reduce(slot[:], locm[:].rearrange(
                "p (t e) -> p t e", e=E), axis=mybir.AxisListType.X,
                op=mybir.AluOpType.add)
            eidm = rsb.tile([P, T * E], f32, tag="eidm")
            nc.vector.tensor_mul(eidm[:], iota_e[:], maskF[:])
            eid = rsb.tile([P, T], f32, tag="eid")
            nc.vector.tensor_reduce(eid[:], eidm[:].rearrange(
                "p (t e) -> p t e", e=E), axis=mybir.AxisListType.X,
                op=mybir.AluOpType.add)
            kept = rsb.tile([P, T], f32, tag="kept")
            nc.vector.tensor_reduce(kept[:], maskF3, axis=mybir.AxisListType.X,
                                    op=mybir.AluOpType.add)

            # my-expert scatter offsets: valid ? slot : DROP
            sel = rsb.tile([P, T], f32, tag="sel")
            nc.vector.tensor_tensor(sel[:], eid[:],
                                    myexp_sb[:].to_broadcast([P, T]),
                                    op=mybir.AluOpType.is_equal)
            valid = rsb.tile([P, T], f32, tag="valid")
            nc.vector.tensor_mul(valid[:], sel[:], kept[:])
            offs_f = rsb.tile([P, T], f32, tag="offs_f")
            nc.vector.tensor_scalar_add(offs_f[:], slot[:], -float(DROP))
            nc.vector.tensor_mul(offs_f[:], offs_f[:], valid[:])
            nc.vector.tensor_scalar_add(offs_f[:], offs_f[:], float(DROP))
            offs_i = rsb.tile([P, T], i32, tag="offs_i")
            nc.vector.tensor_copy(offs_i[:], offs_f[:])

            # ---- scatter (token id, gate bits) pairs into pr_tbl[slot, 0:2].
            # Indirect-DMA scatter semantics on HW: ONE index per partition;
            # that partition's (contiguous) data row is written at the indexed
            # row.  So scatter one column (128 tokens) per call, with a
            # 2-element [tokid, gate] row each.
            pair_sb = rsb.tile([P, 2 * T], i32, tag="pair_sb")
            pair3 = pair_sb[:].rearrange("p (t two) -> p t two", two=2)
            nc.vector.tensor_copy(pair3[:, :, 0:1], tokid_i[:, :, None])
            nc.vector.tensor_copy(pair3[:, :, 1:2].bitcast(f32),
                                  gval[:, :, None])
            init_sb = rsb.tile([P, 2 * CT], i32, tag="init_sb")
            init3 = init_sb[:].rearrange("p (c two) -> p c two", two=2)
            nc.vector.memset(init_sb[:], 0)
            nc.vector.memset(init3[:, :, 0:1], SENT)
            init_tbl = nc.sync.dma_start(
                out=pr_tbl.rearrange("(c p) two -> p c two", p=P),
                in_=init3)
            scatters = []
            for t in range(T):
                sc = nc.gpsimd.indirect_dma_start(
                    out=pr_tbl[:],
                    out_offset=bass.IndirectOffsetOnAxis(
                        ap=offs_i[:, t:t + 1], axis=0),
                    in_=pair3[:, t, :], in_offset=None,
                    bounds_check=CAP - 1, oob_is_err=False)
                tile.add_dep_helper(sc.ins, init_tbl.ins, reason="init<scat")
                scatters.append(sc)

            # read the table back (slot c = ct*128 + p); scatters write DRAM
            # through physical APs invisible to Tile's tracker -> explicit deps
            tbl3 = pr_tbl.rearrange("(c p) two -> p c two", p=P)
            rb_tok = nc.sync.dma_start(out=tok_sb[:, :, None],
                                       in_=tbl3[:, :, 0:1])
            rb_gate = nc.sync.dma_start(out=gate_sb[:, :, None],
                                        in_=tbl3[:, :, 1:2].bitcast(f32))
            nc.sync.dma_start(out=tok.rearrange("(c p) o -> p c o", p=P),
                              in_=tok_sb[:, :, None])
            for sc in scatters:
                tile.add_dep_helper(rb_tok.ins, sc.ins, reason="scat<rb")
                tile.add_dep_helper(rb_gate.ins, sc.ins, reason="scat<rb")

        # =====================  gather + transpose dispatched tokens  =====
        dsp = ctx.enter_context(tc.tile_pool(name="dispT", bufs=1))
        dispT = [dsp.tile([P, CAP], f32r, tag=f"dispT{k}", name=f"dispT{k}")
                 for k in range(KH)]
        with tc.tile_pool(name="gath", bufs=2) as gth, \
             tc.tile_pool(name="gath_ps", bufs=2, space="PSUM") as gps:
            for ct in range(CT):
                stage = gth.tile([P, H], f32, tag="stage")
                nc.vector.memset(stage[:], 0.0)
                nc.gpsimd.indirect_dma_start(
                    out=stage[:], out_offset=None,
                    in_=x[:],
                    in_offset=bass.IndirectOffsetOnAxis(
                        ap=tok_sb[:, ct:ct + 1], axis=0),
                    bounds_check=S - 1, oob_is_err=False)
                for k in range(KH):
                    pst2 = gps.tile([P, P], f32, tag="pst2")
                    nc.tensor.transpose(out=pst2[:],
                                        in_=stage[:, k * P:(k + 1) * P],
                                        identity=ident[:])
                    nc.vector.tensor_copy(
                        dispT[k][:, ct * P:(ct + 1) * P], pst2[:])

        # =====================  GEMM1 + GELU: h1T[f, c]  ===================
        h1p = ctx.enter_context(tc.tile_pool(name="h1T", bufs=1))
        h1T = [h1p.tile([P, CAP], f32r, tag=f"h1T{f}", name=f"h1T{f}")
               for f in range(KF)]
        with tc.tile_pool(name="w1s", bufs=3) as w1p, \
             tc.tile_pool(name="g1ps", bufs=2, space="PSUM") as g1ps:
            for f in range(KF):
                w1f = w1p.tile([P, KH * P], f32r, tag="w1f")
                nc.sync.dma_start(out=w1f[:].rearrange("p (k c) -> p k c", k=KH),
                                  in_=w1t[f].rearrange("k p c -> p k c"))
                for ch in range(2):
                    ps1 = g1ps.tile([P, 512], f32, tag="ps1")
                    for k in range(KH):
                        nc.tensor.matmul(
                            out=ps1[:],
                            lhsT=w1f[:, k * P:(k + 1) * P],
                            rhs=dispT[k][:, ch * 512:(ch + 1) * 512],
                            start=(k == 0), stop=(k == KH - 1))
                    _gelu(nc, w1p, h1T[f][:, ch * 512:(ch + 1) * 512],
                          ps1[:], b1_sb[:, f:f + 1])

        # =====================  GEMM2 + bias + gate scale  =================
        with tc.tile_pool(name="w2s", bufs=3) as w2p, \
             tc.tile_pool(name="g2ps", bufs=1, space="PSUM") as g2ps, \
             tc.tile_pool(name="eos", bufs=3) as eop:
            for hh in range(2):
                ps2 = [g2ps.tile([P, 512], f32, tag=f"ps2_{c}", name=f"ps2_{c}")
                       for c in range(CT)]
                for f in range(KF):
                    w2f = w2p.tile([P, 512], f32r, tag="w2f")
                    nc.sync.dma_start(
                        out=w2f[:],
                        in_=w2e[f * P:(f + 1) * P, hh * 512:(hh + 1) * 512])
                    for c in range(CT):
                        nc.tensor.matmul(
                            out=ps2[c][:],
                            lhsT=h1T[f][:, c * P:(c + 1) * P],
                            rhs=w2f[:],
                            start=(f == 0), stop=(f == KF - 1))
                for c in range(CT):
                    nc.tensor.matmul(
                        out=ps2[c][:], lhsT=ones_row_r[:],
                        rhs=b2_sb[:, hh * 512:(hh + 1) * 512],
                        start=False, stop=True, skip_group_check=True)
                    eot = eop.tile([P, 512], f32, tag="eot")
                    nc.vector.tensor_scalar_mul(eot[:], ps2[c][:],
                                                gate_sb[:, c:c + 1])
                    nc.sync.dma_start(
                        out=eo[c * P:(c + 1) * P, hh * 512:(hh + 1) * 512],
                        in_=eot[:])


def _gelu(nc, pool, out_ap, ps, bias_col):
    if GELU_MODE == "hw":
        nc.scalar.activation(out_ap, ps,
                             mybir.ActivationFunctionType.Gelu_apprx_tanh,
                             bias=bias_col, scale=1.0)
        return
    # sim fallback: exact tanh-form gelu via explicit ops
    v = pool.tile([P, 512], f32, tag="gelu_v", name="gelu_v")
    u = pool.tile([P, 512], f32, tag="gelu_u", name="gelu_u")
    nc.scalar.activation(v[:], ps, mybir.ActivationFunctionType.Identity,
                         bias=bias_col, scale=1.0)
    nc.vector.tensor_mul(u[:], v[:], v[:])
    nc.vector.tensor_mul(u[:], u[:], v[:])
    nc.vector.tensor_scalar(u[:], u[:], 0.044715, None,
                            op0=mybir.AluOpType.mult)
    nc.vector.tensor_add(u[:], u[:], v[:])
    nc.scalar.activation(u[:], u[:], mybir.ActivationFunctionType.Tanh,
                         scale=0.7978845608028654)
    nc.vector.tensor_scalar_add(u[:], u[:], 1.0)
    nc.vector.tensor_mul(u[:], u[:], v[:])
    nc.vector.tensor_scalar_mul(out_ap, u[:], 0.5)


def _prep_inputs(hidden_states, wg, w1, b1, w2, b2):
    x = np.ascontiguousarray(np.asarray(hidden_states, dtype=np.float32))
    xT = np.ascontiguousarray(x.T)
    wg = np.asarray(wg, dtype=np.float32)
    w1 = np.asarray(w1, dtype=np.float32)
    b1 = np.asarray(b1, dtype=np.float32)
    w2 = np.asarray(w2, dtype=np.float32)
    b2 = np.asarray(b2, dtype=np.float32)
    wgt = np.ascontiguousarray(
        wg.reshape(KH, P, E).transpose(1, 0, 2).reshape(P, KH * E))
    in_maps = []
    for e in range(NC):
        w1t = np.ascontiguousarray(
            w1[e].reshape(KH, P, KF, P).transpose(2, 0, 1, 3))
        in_maps.append({
            "x": x, "xT": xT, "wgt": wgt,
            "w1t": w1t,
            "b1t": np.ascontiguousarray(b1[e].reshape(KF, P)),
            "w2e": np.ascontiguousarray(w2[e]),
            "b2e": np.ascontiguousarray(b2[e].reshape(1, H)),
            "myexp": np.full((P, 1), float(e), dtype=np.float32),
        })
    return in_maps


def _assemble(results):
    out = np.zeros((S, H), dtype=np.float32)
    for e in range(NC):
        r = results[e]
        tok = np.asarray(r["tok"]).reshape(-1)
        eo = np.asarray(r["eo"])
        v = tok < S
        out[tok[v]] = eo[v]
    l_aux = np.float32(np.asarray(results[0]["laux"]).reshape(()))
    exp_counts = np.asarray(results[0]["counts"]).reshape(E).astype(np.int32)
    return out, l_aux, exp_counts


def kernel(hidden_states, wg, w1, b1, w2, b2):
    if "nc" not in _CACHE:
        _CACHE["nc"] = build_nc()
    nc = _CACHE["nc"]
    in_maps = _prep_inputs(hidden_states, wg, w1, b1, w2, b2)
    from concourse.bass_utils import run_bass_kernel_spmd
    res = run_bass_kernel_spmd(nc, in_maps, list(range(NC)))
    return _assemble(res.results)


if __name__ == "__main__":
    # quick local CoreSim check of one core (expert 0)
    pass
